# revision 1
# baseline (speedup 1.0000x reference)
"""CodeWiseAttention kernel for Trainium2 (8 NeuronCores, label-dim sharded).

m[b,n,:] = softmax(label_feature[n] @ x[b].T) @ x[b]

Sharding: label rows N=8922 split across 8 cores (1116/core, padded to 1152);
x replicated. Per core, per batch:
  mm1 (fp32r): S^T[l,n] = xT[e,l].T @ labelT[e,n]     (xT via PE transpose)
  exp on ScalarE: expS = exp(S - 30)                   (constant shift; cancels)
  mm2 (fp32r): Uaug^T[e',n] += xa[l,e'].T @ expS^T[l,n]  accumulated over l,
      where xa has a ones column so row 100 of Uaug = Z = sum_l expS.
  out: PE-transpose Uaug^T -> [n, e'], m = U / Z, DMA out.
"""
import numpy as np
from contextlib import ExitStack

import concourse.tile as tile
from concourse import bacc, mybir
from concourse.bass_utils import run_bass_kernel_spmd

F32 = mybir.dt.float32
F32R = mybir.dt.float32r

B, L, E = 8, 2500, 100
LP = 2520          # L padded with zero rows (zero rows add nothing to U or Z)
N_TOTAL = 8922
NCORES = 8
NS = 1116          # label rows per core (core 7: 1110 real)
NSP = 1152         # padded per-core label rows
LC = 126           # l-chunk rows (even: fp32r ISA needs even innermost counts)
NLC = LP // LC     # 20 l-chunks
NCH = 384          # n-chunk width (moving dim; >=256 keeps fp32r at full rate)
NJ = NSP // NCH    # 3 n-chunks
EA = E + 1         # x augmented with ones column
PSB = 512          # psum bank stride in f32 elements
EXP_BIAS = -30.0

TRACE = False
LAST_RESULT = None

_NC = []


def _build():
    nc = bacc.Bacc("TRN2", target_bir_lowering=False, debug=False)
    xa_d = nc.dram_tensor("xa", [B, LP, EA], F32R, kind="ExternalInput").ap()
    lab_d = nc.dram_tensor("lab", [NSP, E], F32R, kind="ExternalInput").ap()
    idr_d = nc.dram_tensor("idr", [128, 128], F32R, kind="ExternalInput").ap()
    idf_d = nc.dram_tensor("idf", [128, 128], F32, kind="ExternalInput").ap()
    m_d = nc.dram_tensor("m", [B, NSP, E], F32, kind="ExternalOutput").ap()

    with tile.TileContext(nc) as tc, ExitStack() as ctx:
        consts = ctx.enter_context(tc.tile_pool(name="consts", bufs=1))
        lab_pool = ctx.enter_context(tc.tile_pool(name="labp", bufs=2))
        xa_pool = ctx.enter_context(tc.tile_pool(name="xap", bufs=2))
        xt_pool = ctx.enter_context(tc.tile_pool(name="xtp", bufs=2))
        e_pool = ctx.enter_context(tc.tile_pool(name="ep", bufs=3))
        u_pool = ctx.enter_context(tc.tile_pool(name="up", bufs=3))
        o_pool = ctx.enter_context(tc.tile_pool(name="op", bufs=4))
        r_pool = ctx.enter_context(tc.tile_pool(name="rp", bufs=4))
        pstr = ctx.enter_context(tc.tile_pool(name="pstr", bufs=2, space="PSUM"))
        pss = ctx.enter_context(tc.tile_pool(name="pss", bufs=1, space="PSUM"))
        psm = ctx.enter_context(tc.tile_pool(name="psm", bufs=1, space="PSUM"))

        idr_sb = consts.tile([128, 128], F32R)
        nc.sync.dma_start(out=idr_sb[:], in_=idr_d)
        idf_sb = consts.tile([128, 128], F32)
        nc.sync.dma_start(out=idf_sb[:], in_=idf_d)
        bias_sb = consts.tile([128, 1], F32)
        nc.vector.memset(bias_sb[:], EXP_BIAS)

        # labelT [E, NSP] via PE transposes of 128-row label chunks
        labT = consts.tile([E, NSP], F32R)
        for k in range(NSP // 128):
            lsb = lab_pool.tile([128, E], F32R, tag="lab")
            nc.sync.dma_start(out=lsb[:], in_=lab_d[k * 128:(k + 1) * 128, :])
            tp = pstr.tile([128, 128], F32R, tag="tr")
            nc.tensor.transpose(tp[:E, :], lsb[:], idr_sb[:, :])
            nc.vector.tensor_copy(labT[:, k * 128:(k + 1) * 128], tp[:E, :])

        # prologue DMA for batch 0; per-batch DMA for b+1 is issued before
        # batch b's compute so the transfer hides under the c-loop
        xa_tiles = {}
        xa_tiles[0] = xa_pool.tile([LC, NLC, EA], F32R, tag="xa", name="xa_sb0")
        nc.sync.dma_start(
            out=xa_tiles[0][:], in_=xa_d[0].rearrange("(c p) e -> p c e", p=LC)
        )
        for b in range(B):
            xa_sb = xa_tiles.pop(b)
            if b + 1 < B:
                xa_tiles[b + 1] = xa_pool.tile(
                    [LC, NLC, EA], F32R, tag="xa", name=f"xa_sb{b+1}")
                nc.sync.dma_start(
                    out=xa_tiles[b + 1][:],
                    in_=xa_d[b + 1].rearrange("(c p) e -> p c e", p=LC),
                )
            # xT [E, LP] for this batch
            xT = xt_pool.tile([E, LP], F32R, tag="xt")
            for c in range(NLC):
                tp = pstr.tile([128, 128], F32R, tag="tr")
                nc.tensor.transpose(
                    tp[:E, :LC], xa_sb[:, c, 0:E], idr_sb[:LC, :LC]
                )
                nc.vector.tensor_copy(xT[:, c * LC:(c + 1) * LC], tp[:E, :LC])

            # two passes over l-chunks: j in {0,1}, then j=2. Halving the
            # S^T tile lets it double-buffer inside 8 PSUM banks, so
            # mm1(c+1) never waits on exp(c).
            u_sbs = []
            for jlo, jn in ((0, 2), (2, 1)):
                m_ps = psm.tile([EA, 2, PSB], F32, tag="m")
                for c in range(NLC):
                    s_ps = pss.tile([LC, 2, PSB], F32, tag="s")
                    for jj in range(jn):
                        nc.tensor.matmul(
                            s_ps[:, jj, 0:NCH],
                            xT[:, c * LC:(c + 1) * LC],
                            labT[:, (jlo + jj) * NCH:(jlo + jj + 1) * NCH],
                        )
                    e_sb = e_pool.tile([LC, 2, NCH], F32R, tag="e")
                    nc.scalar.activation(
                        e_sb[:, 0:jn, :], s_ps[:, 0:jn, 0:NCH],
                        mybir.ActivationFunctionType.Exp,
                        bias=bias_sb[:LC], scale=1.0,
                    )
                    for jj in range(jn):
                        nc.tensor.matmul(
                            m_ps[:, jj, 0:NCH],
                            xa_sb[:, c, :],
                            e_sb[:, jj, :],
                            start=(c == 0), stop=(c == NLC - 1),
                        )
                u_sb = u_pool.tile([EA, 2, NCH], F32, tag="u")
                nc.vector.tensor_copy(
                    u_sb[:, 0:jn, :], m_ps[:, 0:jn, 0:NCH]
                )
                u_sbs.append(u_sb)

            # out path: U^T -> transpose -> divide by Z -> DMA
            for k in range(NSP // 128):
                j, off = divmod(k * 128, NCH)
                u_src = u_sbs[0][:, j, off:off + 128] if j < 2 else \
                    u_sbs[1][:, 0, off:off + 128]
                tpo = pstr.tile([128, 128], F32, tag="tr")
                nc.tensor.transpose(
                    tpo[:, :EA], u_src, idf_sb[:EA, :EA]
                )
                rz = r_pool.tile([128, 1], F32, tag="r")
                nc.vector.reciprocal(rz[:], tpo[:, E:EA])
                o_sb = o_pool.tile([128, E], F32, tag="o")
                nc.vector.tensor_scalar_mul(o_sb[:], tpo[:, 0:E], rz[:])
                nc.sync.dma_start(
                    out=m_d[b, k * 128:(k + 1) * 128, :], in_=o_sb[:]
                )
    nc.compile()
    return nc


def _get_nc():
    if not _NC:
        _NC.append(_build())
    return _NC[0]


def kernel(x, label_feature):
    global LAST_RESULT
    x = np.ascontiguousarray(np.asarray(x, dtype=np.float32))
    lf = np.ascontiguousarray(np.asarray(label_feature, dtype=np.float32))
    assert x.shape == (B, L, E) and lf.shape == (N_TOTAL, E)

    xa = np.zeros((B, LP, EA), np.float32)
    xa[:, :L, :E] = x
    xa[:, :L, E] = 1.0
    ident = np.eye(128, dtype=np.float32)
    in_maps = []
    for r in range(NCORES):
        lo = r * NS
        hi = min(lo + NS, N_TOTAL)
        shard = np.zeros((NSP, E), np.float32)
        shard[: hi - lo] = lf[lo:hi]
        in_maps.append({"xa": xa, "lab": shard, "idr": ident, "idf": ident})

    nc = _get_nc()
    res = run_bass_kernel_spmd(
        nc, in_maps, core_ids=list(range(NCORES)), trace=TRACE
    )
    LAST_RESULT = res

    out = np.empty((B, N_TOTAL, E), np.float32)
    for r in range(NCORES):
        lo = r * NS
        hi = min(lo + NS, N_TOTAL)
        out[:, lo:hi, :] = res.results[r]["m"][:, : hi - lo, :]
    return out



# revision 2
# speedup vs baseline: 3.1240x; 3.1240x over previous
"""CodeWiseAttention kernel for Trainium2 (8 NeuronCores, label-dim sharded).

m[b,n,:] = softmax(label_feature[n] @ x[b].T) @ x[b]

Sharding: label rows N=8922 split across 8 cores (1116/core; core 7 has
1110 real rows). x replicated.

v2 design (vs fp32r baseline):
  - mm1 in fp16 (1 cycle/row on PE vs ~3.3 for fp32 HIGH mode); scores
    accumulate in fp32 PSUM. fp16 input rounding keeps score error ~1e-3.
  - exp on ScalarE reads fp32 PSUM, writes bf16 (range needs bf16: e^30).
  - mm2 in bf16 (xa bf16 stationary, expS bf16 moving), fp32 PSUM accum.
  - No on-device input transposes: host supplies xT [E,L] fp16 and
    labT [E,N] fp16 directly; xa pre-chunked [126,20,101] bf16.
  - j-outer loop (3 n-chunks of 372); exp instructions span 3 l-chunks
    (free dim 1116) to amortize ScalarE's ~352-cycle fixed overhead.
  - s_ps double-buffered (2x3 PSUM banks) so PE never stalls on exp.

Per core, per batch, per n-chunk j (372 labels):
  for each group of 3 l-chunks (126 rows each; 7 groups cover 2520):
    mm1 x3: S^T[l,n] = xT[e,l].T @ labT[e,n]          (fp16, PSUM f32)
    exp:    e_sb[l, 3, n] = exp(S - 30) -> bf16        (one ACT instr)
    mm2 x3: U^T[e',n] += xa[l,e'].T @ e_sb[l,n]        (bf16, accum PSUM)
  xa has a ones column so row 100 of U^T = Z = sum_l expS.
  out: copy U^T to SBUF, PE-transpose 124-wide tiles, m = U/Z, DMA out.
"""
import numpy as np
from contextlib import ExitStack

import ml_dtypes

import concourse.tile as tile
from concourse import bacc, mybir
from concourse.bass_utils import run_bass_kernel_spmd

F32 = mybir.dt.float32
F16 = mybir.dt.float16
BF16 = mybir.dt.bfloat16

BF16NP = ml_dtypes.bfloat16

B, L, E = 8, 2500, 100
LP = 2520          # L padded (pad rows: xT cols zero, xa rows zero)
LC = 126           # l-chunk rows
NLC = LP // LC     # 20 l-chunks
CG = 3             # l-chunks per exp group
GROUPS = [CG] * (NLC // CG) + ([NLC % CG] if NLC % CG else [])  # [3]*6+[2]
N_TOTAL = 8922
NCORES = 8
NS = 1116          # label rows per core (core 7: 1110 real); 3*372
NCH = 372          # n-chunk width (>=256 keeps matmul at full rate)
NJ = NS // NCH     # 3 n-chunks
NO = 124           # out-tile rows; 9*124 = 1116
EA = E + 1         # x augmented with ones column
PSB = 512          # psum bank stride in f32 elements
EXP_BIAS = -30.0

TRACE = False
LAST_RESULT = None

_NC = []


def _build():
    nc = bacc.Bacc("TRN2", target_bir_lowering=False, debug=False)
    xt_d = nc.dram_tensor("xt", [B, E, LP], F16, kind="ExternalInput").ap()
    xa_d = nc.dram_tensor("xa", [B, LC, NLC, EA], BF16, kind="ExternalInput").ap()
    labt_d = nc.dram_tensor("labt", [E, NS], F16, kind="ExternalInput").ap()
    idf_d = nc.dram_tensor("idf", [128, 128], F32, kind="ExternalInput").ap()
    m_d = nc.dram_tensor("m", [B, NS, E], F32, kind="ExternalOutput").ap()

    with tile.TileContext(nc) as tc, ExitStack() as ctx:
        consts = ctx.enter_context(tc.tile_pool(name="consts", bufs=1))
        xt_pool = ctx.enter_context(tc.tile_pool(name="xtp", bufs=2))
        xa_pool = ctx.enter_context(tc.tile_pool(name="xap", bufs=2))
        e_pool = ctx.enter_context(tc.tile_pool(name="ep", bufs=3))
        u_pool = ctx.enter_context(tc.tile_pool(name="up", bufs=2))
        o_pool = ctx.enter_context(tc.tile_pool(name="op", bufs=4))
        r_pool = ctx.enter_context(tc.tile_pool(name="rp", bufs=4))
        pss = ctx.enter_context(tc.tile_pool(name="pss", bufs=2, space="PSUM"))
        psm = ctx.enter_context(tc.tile_pool(name="psm", bufs=1, space="PSUM"))
        pstr = ctx.enter_context(tc.tile_pool(name="pstr", bufs=1, space="PSUM"))

        idf_sb = consts.tile([128, 128], F32)
        nc.sync.dma_start(out=idf_sb[:], in_=idf_d)
        bias_sb = consts.tile([128, 1], F32)
        nc.vector.memset(bias_sb[:], EXP_BIAS)
        labt_sb = consts.tile([E, NS], F16)
        nc.sync.dma_start(out=labt_sb[:], in_=labt_d)

        xt_tiles = {}
        xa_tiles = {}

        def fetch(b):
            xt_tiles[b] = xt_pool.tile([E, LP], F16, tag="xt", name=f"xt{b}")
            nc.sync.dma_start(out=xt_tiles[b][:], in_=xt_d[b])
            xa_tiles[b] = xa_pool.tile([LC, NLC, EA], BF16, tag="xa",
                                       name=f"xa{b}")
            nc.sync.dma_start(out=xa_tiles[b][:], in_=xa_d[b])

        fetch(0)
        for b in range(B):
            xt_sb = xt_tiles.pop(b)
            xa_sb = xa_tiles.pop(b)
            if b + 1 < B:
                fetch(b + 1)
            for j in range(NJ):
                m_ps = psm.tile([EA, PSB], F32, tag="m")
                c = 0
                for cg in GROUPS:
                    s_ps = pss.tile([LC, CG, PSB], F32, tag="s")
                    for k in range(cg):
                        nc.tensor.matmul(
                            s_ps[:, k, 0:NCH],
                            xt_sb[:, (c + k) * LC:(c + k + 1) * LC],
                            labt_sb[:, j * NCH:(j + 1) * NCH],
                        )
                    e_sb = e_pool.tile([LC, CG, NCH], BF16, tag="e")
                    nc.scalar.activation(
                        e_sb[:, 0:cg, :], s_ps[:, 0:cg, 0:NCH],
                        mybir.ActivationFunctionType.Exp,
                        bias=bias_sb[:LC], scale=1.0,
                    )
                    for k in range(cg):
                        nc.tensor.matmul(
                            m_ps[:, 0:NCH],
                            xa_sb[:, c + k, :],
                            e_sb[:, k, :],
                            start=(c + k == 0), stop=(c + k == NLC - 1),
                        )
                    c += cg

                u_sb = u_pool.tile([EA, NCH], F32, tag="u")
                nc.vector.tensor_copy(u_sb[:], m_ps[:, 0:NCH])
                for t in range(NCH // NO):
                    tpo = pstr.tile([128, 128], F32, tag="tr")
                    nc.tensor.transpose(
                        tpo[:NO, :EA], u_sb[:, t * NO:(t + 1) * NO],
                        idf_sb[:EA, :EA],
                    )
                    rz = r_pool.tile([NO, 1], F32, tag="r")
                    nc.vector.reciprocal(rz[:], tpo[:NO, E:EA])
                    o_sb = o_pool.tile([NO, E], F32, tag="o")
                    nc.vector.tensor_scalar_mul(o_sb[:], tpo[:NO, 0:E], rz[:])
                    n0 = j * NCH + t * NO
                    nc.sync.dma_start(out=m_d[b, n0:n0 + NO, :], in_=o_sb[:])
    nc.compile()
    return nc


def _get_nc():
    if not _NC:
        _NC.append(_build())
    return _NC[0]


def kernel(x, label_feature):
    global LAST_RESULT
    x = np.ascontiguousarray(np.asarray(x, dtype=np.float32))
    lf = np.ascontiguousarray(np.asarray(label_feature, dtype=np.float32))
    assert x.shape == (B, L, E) and lf.shape == (N_TOTAL, E)

    # xT [B, E, LP] fp16 (mm1 stationary source; pad cols zero)
    xt = np.zeros((B, E, LP), np.float16)
    xt[:, :, :L] = x.transpose(0, 2, 1)
    # xa [B, LP, EA] bf16 with ones column, pre-chunked to [B, LC, NLC, EA]
    xa_full = np.zeros((B, LP, EA), np.float32)
    xa_full[:, :L, :E] = x
    xa_full[:, :L, E] = 1.0
    xa = np.ascontiguousarray(
        xa_full.reshape(B, NLC, LC, EA).transpose(0, 2, 1, 3)
    ).astype(BF16NP)
    ident = np.eye(128, dtype=np.float32)

    in_maps = []
    for r in range(NCORES):
        lo = r * NS
        hi = min(lo + NS, N_TOTAL)
        shard = np.zeros((NS, E), np.float32)
        shard[: hi - lo] = lf[lo:hi]
        labt = np.ascontiguousarray(shard.T).astype(np.float16)
        in_maps.append({"xt": xt, "xa": xa, "labt": labt, "idf": ident})

    nc = _get_nc()
    res = run_bass_kernel_spmd(
        nc, in_maps, core_ids=list(range(NCORES)), trace=TRACE
    )
    LAST_RESULT = res

    out = np.empty((B, N_TOTAL, E), np.float32)
    for r in range(NCORES):
        lo = r * NS
        hi = min(lo + NS, N_TOTAL)
        out[:, lo:hi, :] = res.results[r]["m"][:, : hi - lo, :]
    return out


# revision 4
# speedup vs baseline: 3.1375x; 1.0043x over previous
"""CodeWiseAttention kernel for Trainium2 (8 NeuronCores, label-dim sharded).

m[b,n,:] = softmax(label_feature[n] @ x[b].T) @ x[b]

Sharding: label rows N=8922 split across 8 cores (1116/core; core 7 has
1110 real rows). x replicated.

v2 design (vs fp32r baseline):
  - mm1 in fp16 (1 cycle/row on PE vs ~3.3 for fp32 HIGH mode); scores
    accumulate in fp32 PSUM. fp16 input rounding keeps score error ~1e-3.
  - exp on ScalarE reads fp32 PSUM, writes bf16 (range needs bf16: e^30).
  - mm2 in bf16 (xa bf16 stationary, expS bf16 moving), fp32 PSUM accum.
  - No on-device input transposes: host supplies xT [E,L] fp16 and
    labT [E,N] fp16 directly; xa pre-chunked [126,20,101] bf16.
  - j-outer loop (3 n-chunks of 372); exp instructions span 3 l-chunks
    (free dim 1116) to amortize ScalarE's ~352-cycle fixed overhead.
  - s_ps double-buffered (2x3 PSUM banks) so PE never stalls on exp.

Per core, per batch, per n-chunk j (372 labels):
  for each group of 3 l-chunks (126 rows each; 7 groups cover 2520):
    mm1 x3: S^T[l,n] = xT[e,l].T @ labT[e,n]          (fp16, PSUM f32)
    exp:    e_sb[l, 3, n] = exp(S - 30) -> bf16        (one ACT instr)
    mm2 x3: U^T[e',n] += xa[l,e'].T @ e_sb[l,n]        (bf16, accum PSUM)
  xa has a ones column so row 100 of U^T = Z = sum_l expS.
  out: copy U^T to SBUF, PE-transpose 124-wide tiles, m = U/Z, DMA out.
"""
import numpy as np
from contextlib import ExitStack

import ml_dtypes

import concourse.tile as tile
from concourse import bacc, mybir
from concourse.bass_utils import run_bass_kernel_spmd

F32 = mybir.dt.float32
F16 = mybir.dt.float16
BF16 = mybir.dt.bfloat16

BF16NP = ml_dtypes.bfloat16

B, L, E = 8, 2500, 100
LP = 2520          # L padded (pad rows: xT cols zero, xa rows zero)
LC = 126           # l-chunk rows
NLC = LP // LC     # 20 l-chunks
CG = 3             # l-chunks per exp group
GROUPS = [CG] * (NLC // CG) + ([NLC % CG] if NLC % CG else [])  # [3]*6+[2]
N_TOTAL = 8922
NCORES = 8
NS = 1116          # label rows per core (core 7: 1110 real); 3*372
NCH = 372          # n-chunk width (>=256 keeps matmul at full rate)
NJ = NS // NCH     # 3 n-chunks
NO = 124           # out-tile rows; 9*124 = 1116
EA = E + 1         # x augmented with ones column
PSB = 512          # psum bank stride in f32 elements
EXP_BIAS = -30.0

TRACE = False
LAST_RESULT = None

_NC = []


def _build():
    nc = bacc.Bacc("TRN2", target_bir_lowering=False, debug=False)
    xt_d = nc.dram_tensor("xt", [B, E, LP], F16, kind="ExternalInput").ap()
    xa_d = nc.dram_tensor("xa", [B, LC, NLC, EA], BF16, kind="ExternalInput").ap()
    labt_d = nc.dram_tensor("labt", [E, NS], F16, kind="ExternalInput").ap()
    idf_d = nc.dram_tensor("idf", [128, 128], F32, kind="ExternalInput").ap()
    m_d = nc.dram_tensor("m", [B, NS, E], F32, kind="ExternalOutput").ap()

    with tile.TileContext(nc) as tc, ExitStack() as ctx:
        consts = ctx.enter_context(tc.tile_pool(name="consts", bufs=1))
        xt_pool = ctx.enter_context(tc.tile_pool(name="xtp", bufs=2))
        xa_pool = ctx.enter_context(tc.tile_pool(name="xap", bufs=2))
        e_pool = ctx.enter_context(tc.tile_pool(name="ep", bufs=4))
        u_pool = ctx.enter_context(tc.tile_pool(name="up", bufs=2))
        o_pool = ctx.enter_context(tc.tile_pool(name="op", bufs=4))
        r_pool = ctx.enter_context(tc.tile_pool(name="rp", bufs=4))
        pss = ctx.enter_context(tc.tile_pool(name="pss", bufs=2, space="PSUM"))
        psm = ctx.enter_context(tc.tile_pool(name="psm", bufs=1, space="PSUM"))
        pstr = ctx.enter_context(tc.tile_pool(name="pstr", bufs=1, space="PSUM"))

        idf_sb = consts.tile([128, 128], F32)
        nc.sync.dma_start(out=idf_sb[:], in_=idf_d)
        bias_sb = consts.tile([128, 1], F32)
        nc.vector.memset(bias_sb[:], EXP_BIAS)
        labt_sb = consts.tile([E, NS], F16)
        nc.sync.dma_start(out=labt_sb[:], in_=labt_d)

        xt_tiles = {}
        xa_tiles = {}

        def fetch(b):
            xt_tiles[b] = xt_pool.tile([E, LP], F16, tag="xt", name=f"xt{b}")
            nc.sync.dma_start(out=xt_tiles[b][:], in_=xt_d[b])
            xa_tiles[b] = xa_pool.tile([LC, NLC, EA], BF16, tag="xa",
                                       name=f"xa{b}")
            nc.sync.dma_start(out=xa_tiles[b][:], in_=xa_d[b])

        # Out-path work for the just-finished (b, j) is interleaved into the
        # NEXT j's groups so its PE transposes never sit in the PE queue
        # ahead of the next mm1 block (which would stall the exp pipeline).
        pending = []   # list of closures, one 124-wide out tile each

        def out_path(b, j, u_sb, t):
            def emit():
                tpo = pstr.tile([128, 128], F32, tag="tr")
                nc.tensor.transpose(
                    tpo[:NO, :EA], u_sb[:, t * NO:(t + 1) * NO],
                    idf_sb[:EA, :EA],
                )
                rz = r_pool.tile([NO, 1], F32, tag="r")
                nc.vector.reciprocal(rz[:], tpo[:NO, E:EA])
                o_sb = o_pool.tile([NO, E], F32, tag="o")
                nc.vector.tensor_scalar_mul(o_sb[:], tpo[:NO, 0:E], rz[:])
                n0 = j * NCH + t * NO
                nc.sync.dma_start(out=m_d[b, n0:n0 + NO, :], in_=o_sb[:])
            return emit

        fetch(0)
        for b in range(B):
            xt_sb = xt_tiles.pop(b)
            xa_sb = xa_tiles.pop(b)
            if b + 1 < B:
                fetch(b + 1)
            for j in range(NJ):
                m_ps = psm.tile([EA, PSB], F32, tag="m")
                c = 0
                for cg in GROUPS:
                    s_ps = pss.tile([LC, CG, PSB], F32, tag="s")
                    for k in range(cg):
                        nc.tensor.matmul(
                            s_ps[:, k, 0:NCH],
                            xt_sb[:, (c + k) * LC:(c + k + 1) * LC],
                            labt_sb[:, j * NCH:(j + 1) * NCH],
                        )
                    e_sb = e_pool.tile([LC, CG, NCH], BF16, tag="e")
                    nc.scalar.activation(
                        e_sb[:, 0:cg, :], s_ps[:, 0:cg, 0:NCH],
                        mybir.ActivationFunctionType.Exp,
                        bias=bias_sb[:LC], scale=1.0,
                    )
                    for k in range(cg):
                        nc.tensor.matmul(
                            m_ps[:, 0:NCH],
                            xa_sb[:, c + k, :],
                            e_sb[:, k, :],
                            start=(c + k == 0), stop=(c + k == NLC - 1),
                        )
                    c += cg
                    if pending:
                        pending.pop(0)()

                u_sb = u_pool.tile([EA, NCH], F32, tag="u")
                nc.vector.tensor_copy(u_sb[:], m_ps[:, 0:NCH])
                pending.extend(out_path(b, j, u_sb, t) for t in range(NCH // NO))
        for p in pending:
            p()
    nc.compile()
    return nc


def _get_nc():
    if not _NC:
        _NC.append(_build())
    return _NC[0]


def kernel(x, label_feature):
    global LAST_RESULT
    x = np.ascontiguousarray(np.asarray(x, dtype=np.float32))
    lf = np.ascontiguousarray(np.asarray(label_feature, dtype=np.float32))
    assert x.shape == (B, L, E) and lf.shape == (N_TOTAL, E)

    # xT [B, E, LP] fp16 (mm1 stationary source; pad cols zero)
    xt = np.zeros((B, E, LP), np.float16)
    xt[:, :, :L] = x.transpose(0, 2, 1)
    # xa [B, LP, EA] bf16 with ones column, pre-chunked to [B, LC, NLC, EA]
    xa_full = np.zeros((B, LP, EA), np.float32)
    xa_full[:, :L, :E] = x
    xa_full[:, :L, E] = 1.0
    xa = np.ascontiguousarray(
        xa_full.reshape(B, NLC, LC, EA).transpose(0, 2, 1, 3)
    ).astype(BF16NP)
    ident = np.eye(128, dtype=np.float32)

    in_maps = []
    for r in range(NCORES):
        lo = r * NS
        hi = min(lo + NS, N_TOTAL)
        shard = np.zeros((NS, E), np.float32)
        shard[: hi - lo] = lf[lo:hi]
        labt = np.ascontiguousarray(shard.T).astype(np.float16)
        in_maps.append({"xt": xt, "xa": xa, "labt": labt, "idf": ident})

    nc = _get_nc()
    res = run_bass_kernel_spmd(
        nc, in_maps, core_ids=list(range(NCORES)), trace=TRACE
    )
    LAST_RESULT = res

    out = np.empty((B, N_TOTAL, E), np.float32)
    for r in range(NCORES):
        lo = r * NS
        hi = min(lo + NS, N_TOTAL)
        out[:, lo:hi, :] = res.results[r]["m"][:, : hi - lo, :]
    return out


# revision 7
# speedup vs baseline: 3.1757x; 1.0122x over previous
"""CodeWiseAttention kernel for Trainium2 (8 NeuronCores, label-dim sharded).

m[b,n,:] = softmax(label_feature[n] @ x[b].T) @ x[b]

Sharding: label rows N=8922 split across 8 cores (1116/core; core 7 has
1110 real rows). x replicated.

v2 design (vs fp32r baseline):
  - mm1 in fp16 (1 cycle/row on PE vs ~3.3 for fp32 HIGH mode); scores
    accumulate in fp32 PSUM. fp16 input rounding keeps score error ~1e-3.
  - exp on ScalarE reads fp32 PSUM, writes bf16 (range needs bf16: e^30).
  - mm2 in bf16 (xa bf16 stationary, expS bf16 moving), fp32 PSUM accum.
  - No on-device input transposes: host supplies xT [E,L] fp16 and
    labT [E,N] fp16 directly; xa pre-chunked [126,20,101] bf16.
  - j-outer loop (3 n-chunks of 372); exp instructions span 3 l-chunks
    (free dim 1116) to amortize ScalarE's ~352-cycle fixed overhead.
  - s_ps double-buffered (2x3 PSUM banks) so PE never stalls on exp.

Per core, per batch, per n-chunk j (372 labels):
  for each group of 3 l-chunks (126 rows each; 7 groups cover 2520):
    mm1 x3: S^T[l,n] = xT[e,l].T @ labT[e,n]          (fp16, PSUM f32)
    exp:    e_sb[l, 3, n] = exp(S - 30) -> bf16        (one ACT instr)
    mm2 x3: U^T[e',n] += xa[l,e'].T @ e_sb[l,n]        (bf16, accum PSUM)
  xa has a ones column so row 100 of U^T = Z = sum_l expS.
  out: copy U^T to SBUF, PE-transpose 124-wide tiles, m = U/Z, DMA out.
"""
import numpy as np
from contextlib import ExitStack

import ml_dtypes

import concourse.tile as tile
from concourse import bacc, mybir
from concourse.bass_utils import run_bass_kernel_spmd

F32 = mybir.dt.float32
F16 = mybir.dt.float16
BF16 = mybir.dt.bfloat16

BF16NP = ml_dtypes.bfloat16

B, L, E = 8, 2500, 100
LP = 2520          # L padded (pad rows: xT cols zero, xa rows zero)
LC = 126           # l-chunk rows
NLC = LP // LC     # 20 l-chunks
CG = 3             # l-chunks per exp group
# short group FIRST: the boundary exp (last group of each j) must be a
# full 1116-wide instruction so it covers the PE chain mm2(last)+mm1(next)
# that gates the next j's first exp; a trailing 744-wide exp leaves a
# ~480ns ScalarE bubble at every j boundary.
GROUPS = [2] + [CG] * 6    # 2+18 = 20 l-chunks
HEADC = 2          # l-chunks in the head DMA piece (covers group 0)
N_TOTAL = 8922
NCORES = 8
NS = 1116          # label rows per core (core 7: 1110 real); 3*372
NCH = 372          # n-chunk width (>=256 keeps matmul at full rate)
NJ = NS // NCH     # 3 n-chunks
NO = 124           # out-tile rows; 9*124 = 1116
EA = E + 1         # x augmented with ones column
PSB = 512          # psum bank stride in f32 elements
EXP_BIAS = -30.0

TRACE = False
LAST_RESULT = None

_NC = []


def _build():
    nc = bacc.Bacc("TRN2", target_bir_lowering=False, debug=False)
    xt_d = nc.dram_tensor("xt", [B, E, LP], F16, kind="ExternalInput").ap()
    xa_d = nc.dram_tensor("xa", [B, LC, NLC, EA], BF16, kind="ExternalInput").ap()
    labt_d = nc.dram_tensor("labt", [E, NS], F16, kind="ExternalInput").ap()
    idf_d = nc.dram_tensor("idf", [128, 128], F32, kind="ExternalInput").ap()
    m_d = nc.dram_tensor("m", [B, NS, E], F32, kind="ExternalOutput").ap()

    with tile.TileContext(nc) as tc, ExitStack() as ctx:
        consts = ctx.enter_context(tc.tile_pool(name="consts", bufs=1))
        xt_pool = ctx.enter_context(tc.tile_pool(name="xtp", bufs=2))
        xa_pool = ctx.enter_context(tc.tile_pool(name="xap", bufs=2))
        e_pool = ctx.enter_context(tc.tile_pool(name="ep", bufs=4))
        u_pool = ctx.enter_context(tc.tile_pool(name="up", bufs=2))
        o_pool = ctx.enter_context(tc.tile_pool(name="op", bufs=4))
        r_pool = ctx.enter_context(tc.tile_pool(name="rp", bufs=4))
        pss = ctx.enter_context(tc.tile_pool(name="pss", bufs=2, space="PSUM"))
        psm = ctx.enter_context(tc.tile_pool(name="psm", bufs=1, space="PSUM"))
        pstr = ctx.enter_context(tc.tile_pool(name="pstr", bufs=1, space="PSUM"))

        bias_sb = consts.tile([128, 1], F32)
        nc.vector.memset(bias_sb[:], EXP_BIAS)
        labt_sb = consts.tile([E, NS], F16)
        nc.sync.dma_start(out=labt_sb[:], in_=labt_d)

        HL = HEADC * LC    # head columns of xT
        xt_tiles = {}
        xa_tiles = {}

        # head/tail DMA split so batch 0's first mm1/mm2 can start after
        # ~100KB lands instead of the full ~1MB (cuts ~7us off the fill)
        def fetch(b):
            xth = xt_pool.tile([E, HL], F16, tag="xth", name=f"xth{b}")
            nc.sync.dma_start(out=xth[:], in_=xt_d[b, :, 0:HL])
            xah = xa_pool.tile([LC, HEADC, EA], BF16, tag="xah",
                              name=f"xah{b}")
            nc.sync.dma_start(out=xah[:], in_=xa_d[b, :, 0:HEADC, :])
            xtt = xt_pool.tile([E, LP - HL], F16, tag="xtt", name=f"xtt{b}")
            nc.sync.dma_start(out=xtt[:], in_=xt_d[b, :, HL:LP])
            xat = xa_pool.tile([LC, NLC - HEADC, EA], BF16, tag="xat",
                              name=f"xat{b}")
            nc.sync.dma_start(out=xat[:], in_=xa_d[b, :, HEADC:NLC, :])
            xt_tiles[b] = (xth, xtt)
            xa_tiles[b] = (xah, xat)

        def xt_col(tiles, c):
            if c < HEADC:
                return tiles[0][:, c * LC:(c + 1) * LC]
            return tiles[1][:, (c - HEADC) * LC:(c - HEADC + 1) * LC]

        def xa_row(tiles, c):
            if c < HEADC:
                return tiles[0][:, c, :]
            return tiles[1][:, c - HEADC, :]

        # Out-path work for the just-finished (b, j) is interleaved into the
        # NEXT j's groups so its PE transposes never sit in the PE queue
        # ahead of the next mm1 block (which would stall the exp pipeline).
        pending = []   # list of closures, one 124-wide out tile each

        def out_path(b, j, u_sb, t):
            def emit():
                tpo = pstr.tile([128, 128], F32, tag="tr")
                nc.tensor.transpose(
                    tpo[:NO, :EA], u_sb[:, t * NO:(t + 1) * NO],
                    idf_sb[:EA, :EA],
                )
                rz = r_pool.tile([NO, 1], F32, tag="r")
                nc.vector.reciprocal(rz[:], tpo[:NO, E:EA])
                o_sb = o_pool.tile([NO, E], F32, tag="o")
                nc.vector.tensor_scalar_mul(o_sb[:], tpo[:NO, 0:E], rz[:])
                n0 = j * NCH + t * NO
                nc.sync.dma_start(out=m_d[b, n0:n0 + NO, :], in_=o_sb[:])
            return emit

        fetch(0)
        idf_sb = consts.tile([128, 128], F32)
        nc.sync.dma_start(out=idf_sb[:], in_=idf_d)
        for b in range(B):
            xt_sb = xt_tiles.pop(b)
            xa_sb = xa_tiles.pop(b)
            if b + 1 < B:
                fetch(b + 1)
            for j in range(NJ):
                m_ps = psm.tile([EA, PSB], F32, tag="m")
                c = 0
                for cg in GROUPS:
                    s_ps = pss.tile([LC, CG, PSB], F32, tag="s")
                    for k in range(cg):
                        nc.tensor.matmul(
                            s_ps[:, k, 0:NCH],
                            xt_col(xt_sb, c + k),
                            labt_sb[:, j * NCH:(j + 1) * NCH],
                        )
                    e_sb = e_pool.tile([LC, CG, NCH], BF16, tag="e")
                    nc.scalar.activation(
                        e_sb[:, 0:cg, :], s_ps[:, 0:cg, 0:NCH],
                        mybir.ActivationFunctionType.Exp,
                        bias=bias_sb[:LC], scale=1.0,
                    )
                    for k in range(cg):
                        nc.tensor.matmul(
                            m_ps[:, 0:NCH],
                            xa_row(xa_sb, c + k),
                            e_sb[:, k, :],
                            start=(c + k == 0), stop=(c + k == NLC - 1),
                        )
                    c += cg
                    if pending:
                        pending.pop(0)()

                u_sb = u_pool.tile([EA, NCH], F32, tag="u")
                nc.vector.tensor_copy(u_sb[:], m_ps[:, 0:NCH])
                pending.extend(out_path(b, j, u_sb, t) for t in range(NCH // NO))
        for p in pending:
            p()
    nc.compile()
    return nc


def _get_nc():
    if not _NC:
        _NC.append(_build())
    return _NC[0]


def kernel(x, label_feature):
    global LAST_RESULT
    x = np.ascontiguousarray(np.asarray(x, dtype=np.float32))
    lf = np.ascontiguousarray(np.asarray(label_feature, dtype=np.float32))
    assert x.shape == (B, L, E) and lf.shape == (N_TOTAL, E)

    # xT [B, E, LP] fp16 (mm1 stationary source; pad cols zero)
    xt = np.zeros((B, E, LP), np.float16)
    xt[:, :, :L] = x.transpose(0, 2, 1)
    # xa [B, LP, EA] bf16 with ones column, pre-chunked to [B, LC, NLC, EA]
    xa_full = np.zeros((B, LP, EA), np.float32)
    xa_full[:, :L, :E] = x
    xa_full[:, :L, E] = 1.0
    xa = np.ascontiguousarray(
        xa_full.reshape(B, NLC, LC, EA).transpose(0, 2, 1, 3)
    ).astype(BF16NP)
    ident = np.eye(128, dtype=np.float32)

    in_maps = []
    for r in range(NCORES):
        lo = r * NS
        hi = min(lo + NS, N_TOTAL)
        shard = np.zeros((NS, E), np.float32)
        shard[: hi - lo] = lf[lo:hi]
        labt = np.ascontiguousarray(shard.T).astype(np.float16)
        in_maps.append({"xt": xt, "xa": xa, "labt": labt, "idf": ident})

    nc = _get_nc()
    res = run_bass_kernel_spmd(
        nc, in_maps, core_ids=list(range(NCORES)), trace=TRACE
    )
    LAST_RESULT = res

    out = np.empty((B, N_TOTAL, E), np.float32)
    for r in range(NCORES):
        lo = r * NS
        hi = min(lo + NS, N_TOTAL)
        out[:, lo:hi, :] = res.results[r]["m"][:, : hi - lo, :]
    return out


# revision 12
# speedup vs baseline: 3.4976x; 1.1014x over previous
"""CodeWiseAttention kernel for Trainium2 (8 NeuronCores, label-dim sharded).

m[b,n,:] = softmax(label_feature[n] @ x[b].T) @ x[b]

Sharding: label rows N=8922 split across 8 cores (1116/core; core 7 has
1110 real rows). x replicated.

v2 design (vs fp32r baseline):
  - mm1 in fp16 (1 cycle/row on PE vs ~3.3 for fp32 HIGH mode); scores
    accumulate in fp32 PSUM. fp16 input rounding keeps score error ~1e-3.
  - exp on ScalarE reads fp32 PSUM, writes bf16 (range needs bf16: e^30).
  - mm2 in bf16 (xa bf16 stationary, expS bf16 moving), fp32 PSUM accum.
  - No on-device input transposes: host supplies xT [E,L] fp16 and
    labT [E,N] fp16 directly; xa pre-chunked [126,20,101] bf16.
  - j-outer loop (3 n-chunks of 372); exp instructions span 3 l-chunks
    (free dim 1116) to amortize ScalarE's ~352-cycle fixed overhead.
  - s_ps double-buffered (2x3 PSUM banks) so PE never stalls on exp.

Per core, per batch, per n-chunk j (372 labels):
  for each group of 3 l-chunks (126 rows each; 7 groups cover 2520):
    mm1 x3: S^T[l,n] = xT[e,l].T @ labT[e,n]          (fp16, PSUM f32)
    exp:    e_sb[l, 3, n] = exp(S - 30) -> bf16        (one ACT instr)
    mm2 x3: U^T[e',n] += xa[l,e'].T @ e_sb[l,n]        (bf16, accum PSUM)
  xa has a ones column so row 100 of U^T = Z = sum_l expS.
  out: copy U^T to SBUF, PE-transpose 124-wide tiles, m = U/Z, DMA out.
"""
import numpy as np
from contextlib import ExitStack

import ml_dtypes

import concourse.tile as tile
from concourse import bacc, mybir
from concourse.bass_utils import run_bass_kernel_spmd

F32 = mybir.dt.float32
F16 = mybir.dt.float16
BF16 = mybir.dt.bfloat16

BF16NP = ml_dtypes.bfloat16

B, L, E = 8, 2500, 100
LP = 2520          # L padded (pad rows: xT cols zero, xa rows zero)
LC = 126           # l-chunk rows
NLC = LP // LC     # 20 l-chunks
CG = 3             # l-chunks per exp group
# short group FIRST: the boundary exp (last group of each j) must be a
# full 1116-wide instruction so it covers the PE chain mm2(last)+mm1(next)
# that gates the next j's first exp; a trailing 744-wide exp leaves a
# ~480ns ScalarE bubble at every j boundary.
GROUPS = [2] + [CG] * 6    # 2+18 = 20 l-chunks
HEADC = 2          # l-chunks in the head DMA piece (covers group 0)
N_TOTAL = 8922
NCORES = 8
NS = 1116          # label rows per core (core 7: 1110 real); 3*372
NCH = 372          # n-chunk width (>=256 keeps matmul at full rate)
NJ = NS // NCH     # 3 n-chunks
NO = 124           # out-tile rows; 9*124 = 1116
EA = E + 1         # x augmented with ones column
PSB = 512          # psum bank stride in f32 elements
EXP_BIAS = -30.0

TRACE = False
LAST_RESULT = None

_NC = []


def _build():
    nc = bacc.Bacc("TRN2", target_bir_lowering=False, debug=False)
    xt_d = nc.dram_tensor("xt", [B, E, LP], F16, kind="ExternalInput").ap()
    xa_d = nc.dram_tensor("xa", [B, LC, NLC, EA], BF16, kind="ExternalInput").ap()
    labt_d = nc.dram_tensor("labt", [E, NS], F16, kind="ExternalInput").ap()
    idf_d = nc.dram_tensor("idf", [128, 128], F32, kind="ExternalInput").ap()
    m_d = nc.dram_tensor("m", [B, NS, E], F32, kind="ExternalOutput").ap()

    with tile.TileContext(nc) as tc, ExitStack() as ctx:
        consts = ctx.enter_context(tc.tile_pool(name="consts", bufs=1))
        xt_pool = ctx.enter_context(tc.tile_pool(name="xtp", bufs=3))
        xa_pool = ctx.enter_context(tc.tile_pool(name="xap", bufs=3))
        e_pool = ctx.enter_context(tc.tile_pool(name="ep", bufs=5))
        u_pool = ctx.enter_context(tc.tile_pool(name="up", bufs=2))
        o_pool = ctx.enter_context(tc.tile_pool(name="op", bufs=4))
        r_pool = ctx.enter_context(tc.tile_pool(name="rp", bufs=4))
        pss = ctx.enter_context(tc.tile_pool(name="pss", bufs=2, space="PSUM"))
        psm = ctx.enter_context(tc.tile_pool(name="psm", bufs=1, space="PSUM"))
        pstr = ctx.enter_context(tc.tile_pool(name="pstr", bufs=1, space="PSUM"))

        bias_sb = consts.tile([128, 1], F32)
        nc.vector.memset(bias_sb[:], EXP_BIAS)
        # per-j label tiles: mm1 of (b0, j0) only waits on the first 74KB
        labt_sb = []
        for j in range(NJ):
            lt = consts.tile([E, NCH], F16, name=f"labt{j}")
            nc.sync.dma_start(out=lt[:], in_=labt_d[:, j * NCH:(j + 1) * NCH])
            labt_sb.append(lt)

        HL = HEADC * LC    # head columns of xT
        xt_tiles = {}
        xa_tiles = {}

        # head/tail DMA split so batch 0's first mm1/mm2 can start after
        # ~100KB lands instead of the full ~1MB (cuts ~7us off the fill)
        def fetch(b):
            xth = xt_pool.tile([E, HL], F16, tag="xth", name=f"xth{b}")
            nc.sync.dma_start(out=xth[:], in_=xt_d[b, :, 0:HL])
            xah = xa_pool.tile([LC, HEADC, EA], BF16, tag="xah",
                              name=f"xah{b}")
            nc.sync.dma_start(out=xah[:], in_=xa_d[b, :, 0:HEADC, :])
            xtt = xt_pool.tile([E, LP - HL], F16, tag="xtt", name=f"xtt{b}")
            nc.sync.dma_start(out=xtt[:], in_=xt_d[b, :, HL:LP])
            xat = xa_pool.tile([LC, NLC - HEADC, EA], BF16, tag="xat",
                              name=f"xat{b}")
            nc.sync.dma_start(out=xat[:], in_=xa_d[b, :, HEADC:NLC, :])
            xt_tiles[b] = (xth, xtt)
            xa_tiles[b] = (xah, xat)

        def xt_col(tiles, c):
            if c < HEADC:
                return tiles[0][:, c * LC:(c + 1) * LC]
            return tiles[1][:, (c - HEADC) * LC:(c - HEADC + 1) * LC]

        def xa_row(tiles, c):
            if c < HEADC:
                return tiles[0][:, c, :]
            return tiles[1][:, c - HEADC, :]

        # Out-path work for the just-finished (b, j) is interleaved into the
        # NEXT j's groups so its PE transposes never sit in the PE queue
        # ahead of the next mm1 block (which would stall the exp pipeline).
        pending = []   # list of closures, one 124-wide out tile each

        def out_path(b, j, u_sb, t):
            def emit():
                tpo = pstr.tile([128, 128], F32, tag="tr")
                nc.tensor.transpose(
                    tpo[:NO, :EA], u_sb[:, t * NO:(t + 1) * NO],
                    idf_sb[:EA, :EA],
                )
                rz = r_pool.tile([NO, 1], F32, tag="r")
                nc.vector.reciprocal(rz[:], tpo[:NO, E:EA])
                o_sb = o_pool.tile([NO, E], F32, tag="o")
                nc.vector.tensor_scalar_mul(o_sb[:], tpo[:NO, 0:E], rz[:])
                n0 = j * NCH + t * NO
                nc.sync.dma_start(out=m_d[b, n0:n0 + NO, :], in_=o_sb[:])
            return emit

        fetch(0)
        idf_sb = consts.tile([128, 128], F32)
        nc.sync.dma_start(out=idf_sb[:], in_=idf_d)

        # Flat software pipeline over all (b, j, group) items. The mm2 block
        # of group G is issued TWO groups after its exp: every PE instruction
        # preceding exp(G+1) in program order (and hence inside its semaphore
        # threshold) then completes at least one full exp earlier, so exps
        # chain back-to-back on ScalarE with no exp->mm2->exp serial bubble.
        items = []
        for b in range(B):
            for j in range(NJ):
                c = 0
                for gi, cg in enumerate(GROUPS):
                    items.append((b, j, gi, cg, c))
                    c += cg

        m_ps_cur = [None]   # current j's accumulator psum tile

        def issue_mm2(it2):
            b2, j2, gi2, cg2, c2 = it2
            if gi2 == 0:
                m_ps_cur[0] = psm.tile([EA, PSB], F32, tag="m", name="m_ps")
            m_ps = m_ps_cur[0]
            xa_sb = xa_tiles[b2]
            e_sb = e_tiles.pop((b2, j2, gi2))
            for k in range(cg2):
                nc.tensor.matmul(
                    m_ps[:, 0:NCH],
                    xa_row(xa_sb, c2 + k),
                    e_sb[:, k, :],
                    start=(c2 + k == 0), stop=(c2 + k == NLC - 1),
                )
            if gi2 == len(GROUPS) - 1:
                u_sb = u_pool.tile([EA, NCH], F32, tag="u")
                nc.vector.tensor_copy(u_sb[:], m_ps[:, 0:NCH])
                pending.extend(
                    out_path(b2, j2, u_sb, t) for t in range(NCH // NO))

        e_tiles = {}
        mm2_q = []
        for it in items + [None, None]:
            if it is not None:
                b, j, gi, cg, c = it
                if j == 0 and gi == 0 and b + 1 < B:
                    fetch(b + 1)
                xt_sb = xt_tiles[b]
                s_ps = pss.tile([LC, CG, PSB], F32, tag="s")
                for k in range(cg):
                    nc.tensor.matmul(
                        s_ps[:, k, 0:NCH],
                        xt_col(xt_sb, c + k),
                        labt_sb[j][:, :],
                    )
                e_sb = e_pool.tile([LC, CG, NCH], BF16, tag="e")
                nc.scalar.activation(
                    e_sb[:, 0:cg, :], s_ps[:, 0:cg, 0:NCH],
                    mybir.ActivationFunctionType.Exp,
                    bias=bias_sb[:LC], scale=1.0,
                )
                e_tiles[(b, j, gi)] = e_sb
                mm2_q.append(it)
            if (len(mm2_q) > 2) or (it is None and mm2_q):
                issue_mm2(mm2_q.pop(0))
                if pending:
                    pending.pop(0)()
        for p in pending:
            p()
    nc.compile()
    return nc


def _get_nc():
    if not _NC:
        _NC.append(_build())
    return _NC[0]


def kernel(x, label_feature):
    global LAST_RESULT
    x = np.ascontiguousarray(np.asarray(x, dtype=np.float32))
    lf = np.ascontiguousarray(np.asarray(label_feature, dtype=np.float32))
    assert x.shape == (B, L, E) and lf.shape == (N_TOTAL, E)

    # xT [B, E, LP] fp16 (mm1 stationary source; pad cols zero)
    xt = np.zeros((B, E, LP), np.float16)
    xt[:, :, :L] = x.transpose(0, 2, 1)
    # xa [B, LP, EA] bf16 with ones column, pre-chunked to [B, LC, NLC, EA]
    xa_full = np.zeros((B, LP, EA), np.float32)
    xa_full[:, :L, :E] = x
    xa_full[:, :L, E] = 1.0
    xa = np.ascontiguousarray(
        xa_full.reshape(B, NLC, LC, EA).transpose(0, 2, 1, 3)
    ).astype(BF16NP)
    ident = np.eye(128, dtype=np.float32)

    in_maps = []
    for r in range(NCORES):
        lo = r * NS
        hi = min(lo + NS, N_TOTAL)
        shard = np.zeros((NS, E), np.float32)
        shard[: hi - lo] = lf[lo:hi]
        labt = np.ascontiguousarray(shard.T).astype(np.float16)
        in_maps.append({"xt": xt, "xa": xa, "labt": labt, "idf": ident})

    nc = _get_nc()
    res = run_bass_kernel_spmd(
        nc, in_maps, core_ids=list(range(NCORES)), trace=TRACE
    )
    LAST_RESULT = res

    out = np.empty((B, N_TOTAL, E), np.float32)
    for r in range(NCORES):
        lo = r * NS
        hi = min(lo + NS, N_TOTAL)
        out[:, lo:hi, :] = res.results[r]["m"][:, : hi - lo, :]
    return out


# revision 16
# speedup vs baseline: 3.5558x; 1.0166x over previous
"""CodeWiseAttention kernel for Trainium2 (8 NeuronCores, label-dim sharded).

m[b,n,:] = softmax(label_feature[n] @ x[b].T) @ x[b]

Sharding: label rows N=8922 split across 8 cores (1116/core; core 7 has
1110 real rows). x replicated.

v2 design (vs fp32r baseline):
  - mm1 in fp16 (1 cycle/row on PE vs ~3.3 for fp32 HIGH mode); scores
    accumulate in fp32 PSUM. fp16 input rounding keeps score error ~1e-3.
  - exp on ScalarE reads fp32 PSUM, writes bf16 (range needs bf16: e^30).
  - mm2 in bf16 (xa bf16 stationary, expS bf16 moving), fp32 PSUM accum.
  - No on-device input transposes: host supplies xT [E,L] fp16 and
    labT [E,N] fp16 directly; xa pre-chunked [126,20,101] bf16.
  - j-outer loop (3 n-chunks of 372); exp instructions span 3 l-chunks
    (free dim 1116) to amortize ScalarE's ~352-cycle fixed overhead.
  - s_ps double-buffered (2x3 PSUM banks) so PE never stalls on exp.

Per core, per batch, per n-chunk j (372 labels):
  for each group of 3 l-chunks (126 rows each; 7 groups cover 2520):
    mm1 x3: S^T[l,n] = xT[e,l].T @ labT[e,n]          (fp16, PSUM f32)
    exp:    e_sb[l, 3, n] = exp(S - 30) -> bf16        (one ACT instr)
    mm2 x3: U^T[e',n] += xa[l,e'].T @ e_sb[l,n]        (bf16, accum PSUM)
  xa has a ones column so row 100 of U^T = Z = sum_l expS.
  out: copy U^T to SBUF, PE-transpose 124-wide tiles, m = U/Z, DMA out.
"""
import numpy as np
from contextlib import ExitStack

import ml_dtypes

import concourse.tile as tile
from concourse import bacc, mybir
from concourse.bass_utils import run_bass_kernel_spmd

F32 = mybir.dt.float32
F16 = mybir.dt.float16
BF16 = mybir.dt.bfloat16

BF16NP = ml_dtypes.bfloat16

B, L, E = 8, 2500, 100
LP = 2520          # L padded (pad rows: xT cols zero, xa rows zero)
LC = 126           # l-chunk rows
NLC = LP // LC     # 20 l-chunks
CG = 3             # l-chunks per exp group
# short group FIRST: the boundary exp (last group of each j) must be a
# full 1116-wide instruction so it covers the PE chain mm2(last)+mm1(next)
# that gates the next j's first exp; a trailing 744-wide exp leaves a
# ~480ns ScalarE bubble at every j boundary.
GROUPS = [2] + [CG] * 6    # 2+18 = 20 l-chunks
HEADC = 2          # l-chunks in the head DMA piece (covers group 0)
N_TOTAL = 8922
NCORES = 8
NS = 1116          # label rows per core (core 7: 1110 real); 3*372
NCH = 372          # n-chunk width (>=256 keeps matmul at full rate)
NJ = NS // NCH     # 3 n-chunks
NO = 124           # out-tile rows; 9*124 = 1116
EA = E + 1         # x augmented with ones column
PSB = 512          # psum bank stride in f32 elements
EXP_BIAS = -30.0

TRACE = False
LAST_RESULT = None

_NC = []


def _build():
    nc = bacc.Bacc("TRN2", target_bir_lowering=False, debug=False)
    xt_d = nc.dram_tensor("xt", [B, E, LP], F16, kind="ExternalInput").ap()
    xa_d = nc.dram_tensor("xa", [B, LC, NLC, EA], BF16, kind="ExternalInput").ap()
    labt_d = nc.dram_tensor("labt", [E, NS], F16, kind="ExternalInput").ap()
    idf_d = nc.dram_tensor("idf", [128, 128], F32, kind="ExternalInput").ap()
    m_d = nc.dram_tensor("m", [B, NS, E], F32, kind="ExternalOutput").ap()

    with tile.TileContext(nc) as tc, ExitStack() as ctx:
        consts = ctx.enter_context(tc.tile_pool(name="consts", bufs=1))
        xt_pool = ctx.enter_context(tc.tile_pool(name="xtp", bufs=3))
        xa_pool = ctx.enter_context(tc.tile_pool(name="xap", bufs=3))
        e_pool = ctx.enter_context(tc.tile_pool(name="ep", bufs=5))
        u_pool = ctx.enter_context(tc.tile_pool(name="up", bufs=2))
        o_pool = ctx.enter_context(tc.tile_pool(name="op", bufs=4))
        r_pool = ctx.enter_context(tc.tile_pool(name="rp", bufs=4))
        pss = ctx.enter_context(tc.tile_pool(name="pss", bufs=2, space="PSUM"))
        psm = ctx.enter_context(tc.tile_pool(name="psm", bufs=1, space="PSUM"))
        pstr = ctx.enter_context(tc.tile_pool(name="pstr", bufs=1, space="PSUM"))

        bias_sb = consts.tile([128, 1], F32)
        nc.vector.memset(bias_sb[:], EXP_BIAS)

        HL = HEADC * LC    # head columns of xT
        MIDC = 8           # l-chunks in the mid xT piece
        ML = MIDC * LC
        xt_tiles = {}
        xa_tiles = {}
        labt_sb = []

        # xT in 3 pieces / xa in 2 so batch 0's pipeline starts after the
        # first ~125KB lands instead of the full ~1MB; each DMA costs
        # ~0.7us of descriptor generation on the sync queue, so the
        # critical-path pieces (labt j0, xT head, xa head) are issued first
        def fetch(b):
            xth = xt_pool.tile([E, HL], F16, tag="xth", name=f"xth{b}")
            nc.sync.dma_start(out=xth[:], in_=xt_d[b, :, 0:HL])
            if b == 0:
                lt0 = consts.tile([E, NCH], F16, name="labt0")
                nc.sync.dma_start(out=lt0[:], in_=labt_d[:, 0:NCH])
                labt_sb.append(lt0)
            xah = xa_pool.tile([LC, HEADC, EA], BF16, tag="xah",
                              name=f"xah{b}")
            nc.sync.dma_start(out=xah[:], in_=xa_d[b, :, 0:HEADC, :])
            xtm = xt_pool.tile([E, ML], F16, tag="xtm", name=f"xtm{b}")
            nc.sync.dma_start(out=xtm[:], in_=xt_d[b, :, HL:HL + ML])
            xtt = xt_pool.tile([E, LP - HL - ML], F16, tag="xtt",
                               name=f"xtt{b}")
            nc.sync.dma_start(out=xtt[:], in_=xt_d[b, :, HL + ML:LP])
            xat = xa_pool.tile([LC, NLC - HEADC, EA], BF16, tag="xat",
                              name=f"xat{b}")
            nc.sync.dma_start(out=xat[:], in_=xa_d[b, :, HEADC:NLC, :])
            if b == 0:
                for j in range(1, NJ):
                    lt = consts.tile([E, NCH], F16, name=f"labt{j}")
                    nc.sync.dma_start(
                        out=lt[:], in_=labt_d[:, j * NCH:(j + 1) * NCH])
                    labt_sb.append(lt)
            xt_tiles[b] = (xth, xtm, xtt)
            xa_tiles[b] = (xah, xat)

        def xt_col(tiles, c):
            if c < HEADC:
                return tiles[0][:, c * LC:(c + 1) * LC]
            if c < HEADC + MIDC:
                c -= HEADC
                return tiles[1][:, c * LC:(c + 1) * LC]
            c -= HEADC + MIDC
            return tiles[2][:, c * LC:(c + 1) * LC]

        def xa_row(tiles, c):
            if c < HEADC:
                return tiles[0][:, c, :]
            return tiles[1][:, c - HEADC, :]

        # Out-path work for the just-finished (b, j) is interleaved into the
        # NEXT j's groups so its PE transposes never sit in the PE queue
        # ahead of the next mm1 block (which would stall the exp pipeline).
        pending = []   # list of closures, one 124-wide out tile each

        def out_path(b, j, u_sb, t, pool=None):
            def emit():
                tpo = (pool or pstr).tile([128, 128], F32, tag="tr",
                                          name="tpo")
                nc.tensor.transpose(
                    tpo[:NO, :EA], u_sb[:, t * NO:(t + 1) * NO],
                    idf_sb[:EA, :EA],
                )
                rz = r_pool.tile([NO, 1], F32, tag="r")
                nc.vector.reciprocal(rz[:], tpo[:NO, E:EA])
                o_sb = o_pool.tile([NO, E], F32, tag="o")
                nc.vector.tensor_scalar_mul(o_sb[:], tpo[:NO, 0:E], rz[:])
                n0 = j * NCH + t * NO
                nc.sync.dma_start(out=m_d[b, n0:n0 + NO, :], in_=o_sb[:])
            return emit

        fetch(0)
        idf_sb = consts.tile([128, 128], F32)
        nc.sync.dma_start(out=idf_sb[:], in_=idf_d)

        # Flat software pipeline over all (b, j, group) items. The mm2 block
        # of group G is issued TWO groups after its exp: every PE instruction
        # preceding exp(G+1) in program order (and hence inside its semaphore
        # threshold) then completes at least one full exp earlier, so exps
        # chain back-to-back on ScalarE with no exp->mm2->exp serial bubble.
        items = []
        for b in range(B):
            for j in range(NJ):
                c = 0
                for gi, cg in enumerate(GROUPS):
                    items.append((b, j, gi, cg, c))
                    c += cg

        m_ps_cur = [None]   # current j's accumulator psum tile

        def issue_mm2(it2):
            b2, j2, gi2, cg2, c2 = it2
            if gi2 == 0:
                m_ps_cur[0] = psm.tile([EA, PSB], F32, tag="m", name="m_ps")
            m_ps = m_ps_cur[0]
            xa_sb = xa_tiles[b2]
            e_sb = e_tiles.pop((b2, j2, gi2))
            for k in range(cg2):
                nc.tensor.matmul(
                    m_ps[:, 0:NCH],
                    xa_row(xa_sb, c2 + k),
                    e_sb[:, k, :],
                    start=(c2 + k == 0), stop=(c2 + k == NLC - 1),
                )
            if gi2 == len(GROUPS) - 1:
                u_sb = u_pool.tile([EA, NCH], F32, tag="u")
                nc.vector.tensor_copy(u_sb[:], m_ps[:, 0:NCH])
                pending.extend(
                    out_path(b2, j2, u_sb, t) for t in range(NCH // NO))

        e_tiles = {}
        mm2_q = []
        for it in items + [None, None]:
            if it is not None:
                b, j, gi, cg, c = it
                if j == 0 and gi == 0 and b + 1 < B:
                    fetch(b + 1)
                xt_sb = xt_tiles[b]
                s_ps = pss.tile([LC, CG, PSB], F32, tag="s")
                for k in range(cg):
                    nc.tensor.matmul(
                        s_ps[:, k, 0:NCH],
                        xt_col(xt_sb, c + k),
                        labt_sb[j][:, :],
                    )
                e_sb = e_pool.tile([LC, CG, NCH], BF16, tag="e")
                nc.scalar.activation(
                    e_sb[:, 0:cg, :], s_ps[:, 0:cg, 0:NCH],
                    mybir.ActivationFunctionType.Exp,
                    bias=bias_sb[:LC], scale=1.0,
                )
                e_tiles[(b, j, gi)] = e_sb
                mm2_q.append(it)
            if (len(mm2_q) > 2) or (it is None and mm2_q):
                issue_mm2(mm2_q.pop(0))
                if pending:
                    pending.pop(0)()
        for p in pending:
            p()
    nc.compile()
    return nc


def _get_nc():
    if not _NC:
        _NC.append(_build())
    return _NC[0]


def kernel(x, label_feature):
    global LAST_RESULT
    x = np.ascontiguousarray(np.asarray(x, dtype=np.float32))
    lf = np.ascontiguousarray(np.asarray(label_feature, dtype=np.float32))
    assert x.shape == (B, L, E) and lf.shape == (N_TOTAL, E)

    # xT [B, E, LP] fp16 (mm1 stationary source; pad cols zero)
    xt = np.zeros((B, E, LP), np.float16)
    xt[:, :, :L] = x.transpose(0, 2, 1)
    # xa [B, LP, EA] bf16 with ones column, pre-chunked to [B, LC, NLC, EA]
    xa_full = np.zeros((B, LP, EA), np.float32)
    xa_full[:, :L, :E] = x
    xa_full[:, :L, E] = 1.0
    xa = np.ascontiguousarray(
        xa_full.reshape(B, NLC, LC, EA).transpose(0, 2, 1, 3)
    ).astype(BF16NP)
    ident = np.eye(128, dtype=np.float32)

    in_maps = []
    for r in range(NCORES):
        lo = r * NS
        hi = min(lo + NS, N_TOTAL)
        shard = np.zeros((NS, E), np.float32)
        shard[: hi - lo] = lf[lo:hi]
        labt = np.ascontiguousarray(shard.T).astype(np.float16)
        in_maps.append({"xt": xt, "xa": xa, "labt": labt, "idf": ident})

    nc = _get_nc()
    res = run_bass_kernel_spmd(
        nc, in_maps, core_ids=list(range(NCORES)), trace=TRACE
    )
    LAST_RESULT = res

    out = np.empty((B, N_TOTAL, E), np.float32)
    for r in range(NCORES):
        lo = r * NS
        hi = min(lo + NS, N_TOTAL)
        out[:, lo:hi, :] = res.results[r]["m"][:, : hi - lo, :]
    return out


# revision 18
# speedup vs baseline: 3.7296x; 1.0489x over previous
"""CodeWiseAttention kernel for Trainium2 (8 NeuronCores, label-dim sharded).

m[b,n,:] = softmax(label_feature[n] @ x[b].T) @ x[b]

Sharding: label rows N=8922 split across 8 cores (1116/core; core 7 has
1110 real rows). x replicated.

v2 design (vs fp32r baseline):
  - mm1 in fp16 (1 cycle/row on PE vs ~3.3 for fp32 HIGH mode); scores
    accumulate in fp32 PSUM. fp16 input rounding keeps score error ~1e-3.
  - exp on ScalarE reads fp32 PSUM, writes bf16 (range needs bf16: e^30).
  - mm2 in bf16 (xa bf16 stationary, expS bf16 moving), fp32 PSUM accum.
  - No on-device input transposes: host supplies xT [E,L] fp16 and
    labT [E,N] fp16 directly; xa pre-chunked [126,20,101] bf16.
  - j-outer loop (3 n-chunks of 372); exp instructions span 3 l-chunks
    (free dim 1116) to amortize ScalarE's ~352-cycle fixed overhead.
  - s_ps double-buffered (2x3 PSUM banks) so PE never stalls on exp.

Per core, per batch, per n-chunk j (372 labels):
  for each group of 3 l-chunks (126 rows each; 7 groups cover 2520):
    mm1 x3: S^T[l,n] = xT[e,l].T @ labT[e,n]          (fp16, PSUM f32)
    exp:    e_sb[l, 3, n] = exp(S - 30) -> bf16        (one ACT instr)
    mm2 x3: U^T[e',n] += xa[l,e'].T @ e_sb[l,n]        (bf16, accum PSUM)
  xa has a ones column so row 100 of U^T = Z = sum_l expS.
  out: copy U^T to SBUF, PE-transpose 124-wide tiles, m = U/Z, DMA out.
"""
import numpy as np
from contextlib import ExitStack

import ml_dtypes

import concourse.tile as tile
from concourse import bacc, mybir
from concourse.bass_utils import run_bass_kernel_spmd

F32 = mybir.dt.float32
F16 = mybir.dt.float16
BF16 = mybir.dt.bfloat16

BF16NP = ml_dtypes.bfloat16

B, L, E = 8, 2500, 100
LP = 2520          # L padded (pad rows: xT cols zero, xa rows zero)
LC = 126           # l-chunk rows
NLC = LP // LC     # 20 l-chunks
CG = 3             # l-chunks per exp group
# short group FIRST: the boundary exp (last group of each j) must be a
# full 1116-wide instruction so it covers the PE chain mm2(last)+mm1(next)
# that gates the next j's first exp; a trailing 744-wide exp leaves a
# ~480ns ScalarE bubble at every j boundary.
GROUPS = [2] + [CG] * 6    # 2+18 = 20 l-chunks
HEADC = 2          # l-chunks in the head DMA piece (covers group 0)
N_TOTAL = 8922
NCORES = 8
NS = 1116          # label rows per core (core 7: 1110 real); 3*372
NCH = 372          # n-chunk width (>=256 keeps matmul at full rate)
NJ = NS // NCH     # 3 n-chunks
NO = 124           # out-tile rows; 9*124 = 1116
EA = E + 1         # x augmented with ones column
PSB = 512          # psum bank stride in f32 elements
EXP_BIAS = -30.0

TRACE = False
LAST_RESULT = None

_NC = []


def _build():
    nc = bacc.Bacc("TRN2", target_bir_lowering=False, debug=False)
    xt_d = nc.dram_tensor("xt", [B, E, LP], F16, kind="ExternalInput").ap()
    xa_d = nc.dram_tensor("xa", [B, LC, NLC, EA], BF16, kind="ExternalInput").ap()
    labt_d = nc.dram_tensor("labt", [E, NS], F16, kind="ExternalInput").ap()
    idf_d = nc.dram_tensor("idf", [128, 128], F32, kind="ExternalInput").ap()
    m_d = nc.dram_tensor("m", [B, NS, E], F32, kind="ExternalOutput").ap()

    with tile.TileContext(nc) as tc, ExitStack() as ctx:
        consts = ctx.enter_context(tc.tile_pool(name="consts", bufs=1))
        xt_pool = ctx.enter_context(tc.tile_pool(name="xtp", bufs=3))
        xa_pool = ctx.enter_context(tc.tile_pool(name="xap", bufs=3))
        e_pool = ctx.enter_context(tc.tile_pool(name="ep", bufs=5))
        u_pool = ctx.enter_context(tc.tile_pool(name="up", bufs=2))
        o_pool = ctx.enter_context(tc.tile_pool(name="op", bufs=4))
        r_pool = ctx.enter_context(tc.tile_pool(name="rp", bufs=4))
        pss = ctx.enter_context(tc.tile_pool(name="pss", bufs=2, space="PSUM"))
        psm = ctx.enter_context(tc.tile_pool(name="psm", bufs=1, space="PSUM"))
        pstr = ctx.enter_context(tc.tile_pool(name="pstr", bufs=1, space="PSUM"))

        bias_sb = consts.tile([128, 1], F32)
        nc.vector.memset(bias_sb[:], EXP_BIAS)

        HL = HEADC * LC    # head columns of xT
        MIDC = 8           # l-chunks in the mid xT piece
        ML = MIDC * LC
        xt_tiles = {}
        xa_tiles = {}
        labt_sb = []

        # xT in 3 pieces / xa in 2 so batch 0's pipeline starts after the
        # first ~125KB lands instead of the full ~1MB; each DMA costs
        # ~0.7us of descriptor generation on the sync queue, so the
        # critical-path pieces (labt j0, xT head, xa head) are issued first
        def fetch(b):
            xth = xt_pool.tile([E, HL], F16, tag="xth", name=f"xth{b}")
            nc.sync.dma_start(out=xth[:], in_=xt_d[b, :, 0:HL])
            if b == 0:
                lt0 = consts.tile([E, NCH], F16, name="labt0")
                nc.sync.dma_start(out=lt0[:], in_=labt_d[:, 0:NCH])
                labt_sb.append(lt0)
            xah = xa_pool.tile([LC, HEADC, EA], BF16, tag="xah",
                              name=f"xah{b}")
            nc.sync.dma_start(out=xah[:], in_=xa_d[b, :, 0:HEADC, :])
            xtm = xt_pool.tile([E, ML], F16, tag="xtm", name=f"xtm{b}")
            nc.sync.dma_start(out=xtm[:], in_=xt_d[b, :, HL:HL + ML])
            xtt = xt_pool.tile([E, LP - HL - ML], F16, tag="xtt",
                               name=f"xtt{b}")
            nc.sync.dma_start(out=xtt[:], in_=xt_d[b, :, HL + ML:LP])
            xat = xa_pool.tile([LC, NLC - HEADC, EA], BF16, tag="xat",
                              name=f"xat{b}")
            nc.sync.dma_start(out=xat[:], in_=xa_d[b, :, HEADC:NLC, :])
            if b == 0:
                for j in range(1, NJ):
                    lt = consts.tile([E, NCH], F16, name=f"labt{j}")
                    nc.sync.dma_start(
                        out=lt[:], in_=labt_d[:, j * NCH:(j + 1) * NCH])
                    labt_sb.append(lt)
            xt_tiles[b] = (xth, xtm, xtt)
            xa_tiles[b] = (xah, xat)

        def xt_col(tiles, c):
            if c < HEADC:
                return tiles[0][:, c * LC:(c + 1) * LC]
            if c < HEADC + MIDC:
                c -= HEADC
                return tiles[1][:, c * LC:(c + 1) * LC]
            c -= HEADC + MIDC
            return tiles[2][:, c * LC:(c + 1) * LC]

        def xa_row(tiles, c):
            if c < HEADC:
                return tiles[0][:, c, :]
            return tiles[1][:, c - HEADC, :]

        # Out-path work for the just-finished (b, j) is interleaved into the
        # NEXT j's groups so its PE transposes never sit in the PE queue
        # ahead of the next mm1 block (which would stall the exp pipeline).
        pending = []   # list of closures, one 124-wide out tile each

        def out_path(b, j, u_sb, t, pool=None):
            def emit():
                tpo = (pool or pstr).tile([128, 128], F32, tag="tr",
                                          name="tpo")
                nc.tensor.transpose(
                    tpo[:NO, :EA], u_sb[:, t * NO:(t + 1) * NO],
                    idf_sb[:EA, :EA],
                )
                rz = r_pool.tile([NO, 1], F32, tag="r")
                nc.vector.reciprocal(rz[:], tpo[:NO, E:EA])
                o_sb = o_pool.tile([NO, E], F32, tag="o")
                nc.vector.tensor_scalar_mul(o_sb[:], tpo[:NO, 0:E], rz[:])
                n0 = j * NCH + t * NO
                nc.sync.dma_start(out=m_d[b, n0:n0 + NO, :], in_=o_sb[:])
            return emit

        fetch(0)
        idf_sb = consts.tile([128, 128], F32)
        nc.sync.dma_start(out=idf_sb[:], in_=idf_d)

        # Flat software pipeline over all (b, j, group) items. The mm2 block
        # of group G is issued TWO groups after its exp: every PE instruction
        # preceding exp(G+1) in program order (and hence inside its semaphore
        # threshold) then completes at least one full exp earlier, so exps
        # chain back-to-back on ScalarE with no exp->mm2->exp serial bubble.
        items = []
        for b in range(B):
            for j in range(NJ):
                c = 0
                for gi, cg in enumerate(GROUPS):
                    items.append((b, j, gi, cg, c))
                    c += cg

        m_ps_cur = [None]   # current j's accumulator psum tile

        def issue_mm2(it2):
            b2, j2, gi2, cg2, c2 = it2
            if gi2 == 0:
                m_ps_cur[0] = psm.tile([EA, PSB], F32, tag="m", name="m_ps")
            m_ps = m_ps_cur[0]
            xa_sb = xa_tiles[b2]
            e_sb = e_tiles.pop((b2, j2, gi2))
            for k in range(cg2):
                nc.tensor.matmul(
                    m_ps[:, 0:NCH],
                    xa_row(xa_sb, c2 + k),
                    e_sb[:, k, :],
                    start=(c2 + k == 0), stop=(c2 + k == NLC - 1),
                )
            if gi2 == len(GROUPS) - 1:
                u_sb = u_pool.tile([EA, NCH], F32, tag="u")
                nc.vector.tensor_copy(u_sb[:], m_ps[:, 0:NCH])
                pending.extend(
                    out_path(b2, j2, u_sb, t) for t in range(NCH // NO))

        e_tiles = {}
        mm2_q = []
        for it in items + [None, None, None]:
            if it is not None:
                b, j, gi, cg, c = it
                if j == 0 and gi == 0 and b + 1 < B:
                    fetch(b + 1)
                xt_sb = xt_tiles[b]
                s_ps = pss.tile([LC, CG, PSB], F32, tag="s")
                for k in range(cg):
                    nc.tensor.matmul(
                        s_ps[:, k, 0:NCH],
                        xt_col(xt_sb, c + k),
                        labt_sb[j][:, :],
                    )
                e_sb = e_pool.tile([LC, CG, NCH], BF16, tag="e")
                nc.scalar.activation(
                    e_sb[:, 0:cg, :], s_ps[:, 0:cg, 0:NCH],
                    mybir.ActivationFunctionType.Exp,
                    bias=bias_sb[:LC], scale=1.0,
                )
                e_tiles[(b, j, gi)] = e_sb
                mm2_q.append(it)
            if (len(mm2_q) > 3) or (it is None and mm2_q):
                issue_mm2(mm2_q.pop(0))
                if pending:
                    pending.pop(0)()
        for p in pending:
            p()
    nc.compile()
    return nc


def _get_nc():
    if not _NC:
        _NC.append(_build())
    return _NC[0]


def kernel(x, label_feature):
    global LAST_RESULT
    x = np.ascontiguousarray(np.asarray(x, dtype=np.float32))
    lf = np.ascontiguousarray(np.asarray(label_feature, dtype=np.float32))
    assert x.shape == (B, L, E) and lf.shape == (N_TOTAL, E)

    # xT [B, E, LP] fp16 (mm1 stationary source; pad cols zero)
    xt = np.zeros((B, E, LP), np.float16)
    xt[:, :, :L] = x.transpose(0, 2, 1)
    # xa [B, LP, EA] bf16 with ones column, pre-chunked to [B, LC, NLC, EA]
    xa_full = np.zeros((B, LP, EA), np.float32)
    xa_full[:, :L, :E] = x
    xa_full[:, :L, E] = 1.0
    xa = np.ascontiguousarray(
        xa_full.reshape(B, NLC, LC, EA).transpose(0, 2, 1, 3)
    ).astype(BF16NP)
    ident = np.eye(128, dtype=np.float32)

    in_maps = []
    for r in range(NCORES):
        lo = r * NS
        hi = min(lo + NS, N_TOTAL)
        shard = np.zeros((NS, E), np.float32)
        shard[: hi - lo] = lf[lo:hi]
        labt = np.ascontiguousarray(shard.T).astype(np.float16)
        in_maps.append({"xt": xt, "xa": xa, "labt": labt, "idf": ident})

    nc = _get_nc()
    res = run_bass_kernel_spmd(
        nc, in_maps, core_ids=list(range(NCORES)), trace=TRACE
    )
    LAST_RESULT = res

    out = np.empty((B, N_TOTAL, E), np.float32)
    for r in range(NCORES):
        lo = r * NS
        hi = min(lo + NS, N_TOTAL)
        out[:, lo:hi, :] = res.results[r]["m"][:, : hi - lo, :]
    return out
